# revision 4
# baseline (speedup 1.0000x reference)
"""Trainium2 Bass kernel for a 6-layer post-LN transformer encoder.

Model (per reference):
  h = (x @ Wemb + bemb) * sqrt(D) + posenc
  for l in 6:  h = LN(h + MHA_l(h))   (8 heads, dh=64, softmax over keys)

Sharding: pure data-parallel over batch. B=16 across 8 NeuronCores,
2 batch elements per core, weights replicated, no collectives.

Per-core layout strategy:
  - h kept in BOTH layouts: S-major h_s[p=s%128, sc=s//128, d] (residual+LN)
    and D-major hT[p=d%128, kc=d//128, s] (matmul operand), refreshed each
    layer via PE transposes.
  - Q,K computed D-major (QT/KT), V computed S-major into a per-head padded
    tile Vp[t, h*65+j] whose 65th column per head is ones, so the
    attention matmul attnT' = Vp'.T @ exp(scoresT) yields both the
    numerator rows (0..63) and the softmax denominator (row 64) in one pass.
  - scoresT[t, s] = K_h.T-major matmul; exp fused with mask-bias + 1/8 scale
    on the scalar engine; softmax denominator reciprocal broadcast across
    partitions with gpsimd.partition_broadcast.
  - All matmul operands are float32r (TF32-like 4x-rate fp32 mode); psum and
    LN arithmetic in fp32.
"""
import math

import numpy as np

# -- model constants (hardcoded per contract) --
B, S, F, D, H, L = 16, 1024, 64, 512, 8, 6
DH = D // H          # 64
P = 128              # partitions
NS = S // P          # 8 s-chunks of 128
KC = D // P          # 4 d-chunks of 128
NH = 2               # s-halves of 512 (fp32 moving-operand limit)
NHW = S // NH        # 512
NCORES = 8
BLOC = B // NCORES   # 2
EPS = 1e-6
SQRT_D = float(np.sqrt(np.float32(D)))
SCALE = 1.0 / float(np.sqrt(np.float32(DH)))

_CACHE = {}


def _posenc_np():
    pos = np.arange(S)[:, None].astype(np.float32)
    i = np.arange(D)[None, :].astype(np.float32)
    angle = pos / np.power(10000.0, 2.0 * (i // 2) / np.float32(D)).astype(np.float32)
    angle[:, 0::2] = np.sin(angle[:, 0::2])
    angle[:, 1::2] = np.cos(angle[:, 1::2])
    return angle.astype(np.float32)  # [S, D]


def _build_nc():
    import concourse.bacc as bacc
    import concourse.mybir as mybir
    import concourse.tile as tile
    from concourse.masks import make_identity

    f32 = mybir.dt.float32
    f32r = mybir.dt.float32r
    AF = mybir.ActivationFunctionType
    OP = mybir.AluOpType

    nc = bacc.Bacc("TRN2", target_bir_lowering=False, debug=False)

    # ---- DRAM io ----
    x2 = nc.dram_tensor("x2", [BLOC, S, F], f32, kind="ExternalInput")
    maskT = nc.dram_tensor("maskT", [BLOC, NS, P], f32, kind="ExternalInput")
    wq_d = nc.dram_tensor("wq", [L, D, D], f32, kind="ExternalInput")
    wk_d = nc.dram_tensor("wk", [L, D, D], f32, kind="ExternalInput")
    wv_d = nc.dram_tensor("wv", [L, D, D], f32, kind="ExternalInput")
    wo_d = nc.dram_tensor("wo", [L, D, D], f32, kind="ExternalInput")
    bq_d = nc.dram_tensor("bq", [L, D], f32, kind="ExternalInput")
    bk_d = nc.dram_tensor("bk", [L, D], f32, kind="ExternalInput")
    bv_d = nc.dram_tensor("bv", [L, D], f32, kind="ExternalInput")
    bo_d = nc.dram_tensor("bo", [L, D], f32, kind="ExternalInput")
    gm_d = nc.dram_tensor("gamma", [L, D], f32, kind="ExternalInput")
    bt_d = nc.dram_tensor("beta", [L, D], f32, kind="ExternalInput")
    we_d = nc.dram_tensor("wemb", [F, D], f32, kind="ExternalInput")
    pe_d = nc.dram_tensor("pose", [S, D], f32, kind="ExternalInput")
    out2 = nc.dram_tensor("out2", [BLOC, S, D], f32, kind="ExternalOutput")

    with tile.TileContext(nc) as tc:
        persist = tc.alloc_tile_pool(name="persist", bufs=1)
        wpool = tc.alloc_tile_pool(name="wpool", bufs=1)
        stage = tc.alloc_tile_pool(name="stage", bufs=3)
        tmp = tc.alloc_tile_pool(name="tmp", bufs=3)
        expp = tc.alloc_tile_pool(name="expp", bufs=3)
        pmm = tc.alloc_tile_pool(name="pmm", bufs=3, space="PSUM")
        psc = tc.alloc_tile_pool(name="psc", bufs=3, space="PSUM")
        pat = tc.alloc_tile_pool(name="pat", bufs=2, space="PSUM")

        # ---- persistent state ----
        ident = persist.tile([P, P], f32, name="ident")
        make_identity(nc, ident[:])
        eps_t = persist.tile([P, 1], f32, name="eps_t")
        nc.vector.memset(eps_t[:], EPS)
        wemb_r = persist.tile([F, D], f32r, name="wemb_r")
        xT = persist.tile([F, S], f32r, name="xT")
        QT = persist.tile([P, KC, S], f32r, name="QT")
        KT = persist.tile([P, KC, S], f32r, name="KT")
        Vp = persist.tile([P, NS, H * (DH + 1)], f32r, name="Vp")
        attnT = persist.tile([P, KC, S], f32r, name="attnT")
        h_s = [persist.tile([P, NS, D], f32, name=f"h_s{b}") for b in range(BLOC)]
        hT = [persist.tile([P, KC, S], f32r, name=f"hT{b}") for b in range(BLOC)]
        mask_t = [persist.tile([P, NS], f32, name=f"mask_t{b}") for b in range(BLOC)]

        # ones columns of Vp (written once; per-layer V writes skip them).
        # memset can't emit float32r, so memset f32 then cast-copy.
        ones_t = persist.tile([P, NS * H], f32, name="ones_t")
        nc.vector.memset(ones_t[:], 1.0)
        nc.vector.tensor_copy(
            Vp[:].rearrange("p t (h j) -> p t h j", j=DH + 1)[:, :, :, DH:DH + 1],
            ones_t[:].rearrange("p (t h) -> p t h", h=H)[:, :, :, None])

        # ---- per-layer weight tiles ----
        w_q = wpool.tile([P, KC, D], f32r, name="w_q")
        w_k = wpool.tile([P, KC, D], f32r, name="w_k")
        w_v = wpool.tile([P, KC, D], f32r, name="w_v")
        w_o = wpool.tile([P, KC, D], f32r, name="w_o")
        bq_t = wpool.tile([P, KC], f32, name="bq_t")
        bk_t = wpool.tile([P, KC], f32, name="bk_t")
        bv_bc = wpool.tile([P, D], f32, name="bv_bc")
        bo_bc = wpool.tile([P, D], f32, name="bo_bc")
        gm_bc = wpool.tile([P, D], f32, name="gm_bc")
        bt_bc = wpool.tile([P, D], f32, name="bt_bc")

        def transpose_to_hT(b):
            for kc in range(KC):
                for sc in range(NS):
                    pt = pmm.tile([P, P], f32, name="pt", tag="mm")
                    nc.tensor.transpose(
                        pt[:], h_s[b][:, sc, kc * P:(kc + 1) * P], ident[:])
                    nc.vector.tensor_copy(hT[b][:, kc, sc * P:(sc + 1) * P], pt[:])

        # ---- embedding (both elems) ----
        st0 = stage.tile([F, D], f32, name="st0", tag="stage")
        nc.sync.dma_start(st0[:], we_d[:, :])
        nc.vector.tensor_copy(wemb_r[:], st0[:])
        for b in range(BLOC):
            nc.sync.dma_start(mask_t[b][:], maskT[b].rearrange("c p -> p c"))
            for sc in range(NS):
                x_sb = stage.tile([P, F], f32, name="x_sb", tag="x_sb")
                nc.sync.dma_start(x_sb[:], x2[b, sc * P:(sc + 1) * P, :])
                pxt = pmm.tile([F, P], f32, name="pxt", tag="mm")
                nc.tensor.transpose(pxt[:], x_sb[:], ident[:])
                nc.vector.tensor_copy(xT[:, sc * P:(sc + 1) * P], pxt[:])
            for sc in range(NS):
                pe_t = stage.tile([P, D], f32, name="pe_t", tag="stage")
                nc.sync.dma_start(pe_t[:], pe_d[sc * P:(sc + 1) * P, :])
                pemb = pmm.tile([P, D], f32, name="pemb", tag="mm")
                nc.tensor.matmul(
                    pemb[:], xT[:, sc * P:(sc + 1) * P], wemb_r[:],
                    start=True, stop=True)
                nc.vector.scalar_tensor_tensor(
                    out=h_s[b][:, sc, :], in0=pemb[:], scalar=SQRT_D,
                    in1=pe_t[:], op0=OP.mult, op1=OP.add)
            transpose_to_hT(b)

        # ---- layers ----
        for l in range(L):
            for wd, wt in ((wq_d, w_q), (wk_d, w_k), (wv_d, w_v), (wo_d, w_o)):
                for kc in range(KC):
                    st = stage.tile([P, D], f32, name="st", tag="stage")
                    nc.sync.dma_start(st[:], wd[l, kc * P:(kc + 1) * P, :])
                    nc.vector.tensor_copy(wt[:, kc, :], st[:])
            nc.sync.dma_start(bq_t[:], bq_d[l].rearrange("(c p) -> p c", p=P))
            nc.sync.dma_start(bk_t[:], bk_d[l].rearrange("(c p) -> p c", p=P))
            nc.sync.dma_start(bv_bc[:], bv_d[l][None, :].to_broadcast((P, D)))
            nc.sync.dma_start(bo_bc[:], bo_d[l][None, :].to_broadcast((P, D)))
            nc.sync.dma_start(gm_bc[:], gm_d[l][None, :].to_broadcast((P, D)))
            nc.sync.dma_start(bt_bc[:], bt_d[l][None, :].to_broadcast((P, D)))

            for b in range(BLOC):
                # -- Q, K projections (D-major) --
                for wt, bias_t, OT in ((w_q, bq_t, QT), (w_k, bk_t, KT)):
                    for dc in range(KC):
                        for sh in range(NH):
                            pq = pmm.tile([P, NHW], f32, name="pq", tag="mm")
                            for kc in range(KC):
                                nc.tensor.matmul(
                                    pq[:],
                                    wt[:, kc, dc * P:(dc + 1) * P],
                                    hT[b][:, kc, sh * NHW:(sh + 1) * NHW],
                                    start=(kc == 0), stop=(kc == KC - 1))
                            nc.vector.tensor_scalar_add(
                                OT[:, dc, sh * NHW:(sh + 1) * NHW], pq[:],
                                bias_t[:, dc:dc + 1])
                # -- V projection (S-major, head-padded with ones col) --
                for tcix in range(NS):
                    pv = pmm.tile([P, D], f32, name="pv", tag="mm")
                    for kc in range(KC):
                        nc.tensor.matmul(
                            pv[:],
                            hT[b][:, kc, tcix * P:(tcix + 1) * P],
                            w_v[:, kc, :],
                            start=(kc == 0), stop=(kc == KC - 1))
                    nc.vector.scalar_tensor_tensor(
                        out=Vp[:, tcix, :]
                        .rearrange("p (h j) -> p h j", j=DH + 1)[:, :, 0:DH],
                        in0=pv[:].rearrange("p (h j) -> p h j", j=DH),
                        scalar=1.0,
                        in1=bv_bc[:].rearrange("p (h j) -> p h j", j=DH),
                        op0=OP.mult, op1=OP.add)
                # -- attention, one head at a time --
                for h in range(H):
                    kcq = h // 2
                    po = (h % 2) * DH
                    c0 = h * (DH + 1)
                    for sh in range(NH):
                        pa = pat.tile([DH + 1, NHW], f32, name="pa", tag="at")
                        for tcix in range(NS):
                            ps_t = psc.tile([P, NHW], f32, name="ps_t", tag="sc")
                            nc.tensor.matmul(
                                ps_t[:],
                                KT[po:po + DH, kcq, tcix * P:(tcix + 1) * P],
                                QT[po:po + DH, kcq, sh * NHW:(sh + 1) * NHW],
                                start=True, stop=True)
                            e_t = expp.tile([P, NHW], f32r, name="e_t", tag="e_t")
                            nc.scalar.activation(
                                out=e_t[:], in_=ps_t[:], func=AF.Exp,
                                bias=mask_t[b][:, tcix:tcix + 1], scale=SCALE)
                            nc.tensor.matmul(
                                pa[:], Vp[:, tcix, c0:c0 + DH + 1], e_t[:],
                                start=(tcix == 0), stop=(tcix == NS - 1))
                        recip = tmp.tile([1, NHW], f32, name="recip", tag="recip", bufs=2)
                        nc.vector.reciprocal(recip[:], pa[DH:DH + 1, :])
                        rec_bc = tmp.tile([DH, NHW], f32, name="rec_bc", tag="rec_bc", bufs=2)
                        nc.gpsimd.partition_broadcast(rec_bc[:], recip[:], channels=DH)
                        nc.vector.tensor_mul(
                            attnT[po:po + DH, kcq, sh * NHW:(sh + 1) * NHW],
                            pa[0:DH, :], rec_bc[:])
                # -- output projection + residual + layernorm --
                for sc in range(NS):
                    po_t = pmm.tile([P, D], f32, name="po_t", tag="mm")
                    for kc in range(KC):
                        nc.tensor.matmul(
                            po_t[:],
                            attnT[:, kc, sc * P:(sc + 1) * P],
                            w_o[:, kc, :],
                            start=(kc == 0), stop=(kc == KC - 1))
                    resid = tmp.tile([P, D], f32, name="resid", tag="resid", bufs=2)
                    nc.vector.tensor_add(resid[:], po_t[:], h_s[b][:, sc, :])
                    nc.vector.tensor_add(resid[:], resid[:], bo_bc[:])
                    stats = tmp.tile([P, 6], f32, name="stats", tag="stats")
                    nc.vector.bn_stats(out=stats[:], in_=resid[:])
                    mv = tmp.tile([P, 2], f32, name="mv", tag="mv")
                    nc.vector.bn_aggr(out=mv[:], in_=stats[:])
                    sq = tmp.tile([P, 1], f32, name="sq", tag="sq")
                    nc.scalar.activation(
                        out=sq[:], in_=mv[:, 1:2], func=AF.Sqrt, bias=eps_t[:])
                    rstd = tmp.tile([P, 1], f32, name="rstd", tag="rstd")
                    nc.vector.reciprocal(rstd[:], sq[:])
                    xc = tmp.tile([P, D], f32, name="xc", tag="xc", bufs=2)
                    nc.vector.tensor_scalar_sub(xc[:], resid[:], mv[:, 0:1])
                    nc.vector.scalar_tensor_tensor(
                        out=xc[:], in0=xc[:], scalar=rstd[:],
                        in1=gm_bc[:], op0=OP.mult, op1=OP.mult)
                    nc.vector.tensor_add(h_s[b][:, sc, :], xc[:], bt_bc[:])
                if l < L - 1:
                    transpose_to_hT(b)

        # ---- store ----
        for b in range(BLOC):
            nc.sync.dma_start(
                out2[b].rearrange("(c p) d -> p c d", p=P), h_s[b][:])

        pat.release()
        psc.release()
        pmm.release()
        expp.release()
        tmp.release()
        stage.release()
        wpool.release()
        persist.release()

    nc.compile()
    return nc


def _get_nc():
    if "nc" not in _CACHE:
        _CACHE["nc"] = _build_nc()
    return _CACHE["nc"]


def kernel(x, padding_mask, training, Wemb, bemb, Wq, bq, Wk, bk, Wv, bv,
           Wo, bo, gamma, beta):
    from concourse.bass_utils import run_bass_kernel_spmd

    nc = _get_nc()

    x = np.asarray(x, dtype=np.float32)
    padding_mask = np.asarray(padding_mask, dtype=np.float32)
    pose = _posenc_np() + np.asarray(bemb, np.float32)[None, :] * np.float32(SQRT_D)

    common = {
        "wq": np.ascontiguousarray(Wq, np.float32),
        "wk": np.ascontiguousarray(Wk, np.float32),
        "wv": np.ascontiguousarray(Wv, np.float32),
        "wo": np.ascontiguousarray(Wo, np.float32),
        "bq": np.ascontiguousarray(bq, np.float32),
        "bk": np.ascontiguousarray(bk, np.float32),
        "bv": np.ascontiguousarray(bv, np.float32),
        "bo": np.ascontiguousarray(bo, np.float32),
        "gamma": np.ascontiguousarray(gamma, np.float32),
        "beta": np.ascontiguousarray(beta, np.float32),
        "wemb": np.ascontiguousarray(Wemb, np.float32),
        "pose": np.ascontiguousarray(pose, np.float32),
    }
    in_maps = []
    for c in range(NCORES):
        xs = x[c * BLOC:(c + 1) * BLOC]
        m = padding_mask[c * BLOC:(c + 1) * BLOC, 0, 0, :] * np.float32(-1e9)
        in_maps.append({
            "x2": np.ascontiguousarray(xs),
            "maskT": np.ascontiguousarray(m.reshape(BLOC, NS, P)),
            **common,
        })

    res = run_bass_kernel_spmd(nc, in_maps, core_ids=list(range(NCORES)))
    out = np.concatenate([r["out2"] for r in res.results], axis=0)
    return out
